# revision 11
# baseline (speedup 1.0000x reference)
import math

import numpy as np

# nn_DescLayer: LayerNorm -> x@M^T, x@R^T -> Nk[b,s,i] = sum_{j,g} P[i,j,g] *
# cos(2*pi*k[b,s]/periods[i,j,g]) * xproj[b,s,j]; out = res + Nk.
# Data-parallel over the 1024 (b,s) tokens: 128 tokens per NeuronCore.
#
# Period-major main pass, token-INNER free layout. Per 16-token tile:
#   ang[p,(i,glo,t)] = k_t * 2pi/periods  (one fp16 DVE tt, k broadcast view)
#   sino = Sin(ang + pi/2) on ScalarE, strided output -> (glo,t,i) layout
#   prodx = sino * P_e (one fp16 DVE tt, P pre-tiled on host)
#   per 8-token half: 4 glo-matmuls, stationary = xprep[:, 8 tokens] fp16,
#   accumulate PSUM [8, 512]; out[t',(t,i)] diag t'==t is Nk.
#   Diagonal extracted via PSUM->HBM dump + strided HBM->SBUF gather
#   (flat HBM stride 576 = 512+64 walks the diagonal blocks).
# The xproj multiply rides the matmul stationary; P/2pi/periods prep
# (reciprocals, fp16 casts, layout tiling, gamma/beta folding into M/R)
# is done on host as parameter preprocessing.
#
# Small periods (flat < 2048, i.e. i < 4) need range reduction; handled in a
# token-major pass (partitions = tokens) with the heavy elementwise ops on
# GpSimd: f = (k/p + 1/4) - round(k/p + 1/4), cos = sin(2pi*f).

B, S, D, NB = 2, 512, 64, 8
N_CORES = 8
TOK = (B * S) // N_CORES  # 128 tokens per core
NPER = D * D * NB  # 32768 periods
SMALL = 2048  # flat period idx < SMALL (i<4) needs range reduction
LN_EPS = 1e-5
TWO_PI = 2.0 * math.pi
RND_C = 12582912.0  # 1.5 * 2**23: (u + C) - C == round-to-nearest(u) in f32
TT = 16  # tokens per main tile
NTILE = TOK // TT  # 8
POOL_ANG = (6,)  # tiles whose angle tt runs on GpSimd instead of DVE

_CACHE = {}


def _split_waits(nc, maxw=1):
    """This walrus build rejects instructions carrying more than one sem
    wait. Hoist excess waits onto same-engine NoOps placed immediately
    before the instruction (same engine stream => executes first)."""
    import bass_rust
    import concourse.mybir as mybir

    ctr = [0]
    for f in nc.m.functions:
        for b in f.blocks:
            new_insts = []
            changed = False
            for inst in b.instructions:
                si = inst.sync_info
                waits = list(si.on_wait) if si and si.on_wait else []
                if len(waits) > maxw:
                    keep = waits[-maxw:]
                    extra = waits[:-maxw]
                    for i0 in range(0, len(extra), maxw):
                        ctr[0] += 1
                        nop = bass_rust.InstNoOp(
                            name=f"I-waitsplit-{ctr[0]}",
                            engine=inst.engine,
                            text_hint="waitsplit",
                            sync_info=mybir.SyncInfo(
                                on_wait=extra[i0 : i0 + maxw], on_update=[]
                            ),
                        )
                        new_insts.append(nop)
                    si.on_wait = keep
                    changed = True
                new_insts.append(inst)
            if changed:
                b.instructions = new_insts


def _build_program(split=True):
    import concourse.bass as bass
    import concourse.mybir as mybir
    from concourse.ap import AP
    from concourse.tile import TileContext
    from concourse.vector_clock import ScopedClock, VectorClock

    # --- workaround: walrus rejects >1 sem wait on the Tile tail drain;
    # spread the waits over SP nops (1 each), then issue a bare drain.
    def _drain_and_barrier(self, tick_clock, wait_clock):
        nc = self.nc
        gc = tick_clock.global_clock
        n = len(gc)
        for i in range(n):
            tick = gc[i]
            if tick <= 0:
                continue
            vec = [0] * n
            vec[i] = tick
            nop_inst = nc.sync.nop(nofuse=True, hint=f"drain_wait_{i}")
            wait_clock.add_sem_waits(
                nop_inst.ins, ScopedClock({None: VectorClock(vec)})
            )
        nc.sync.drain()
        nc.all_engine_barrier()
        assert self.sems is not None
        popped = nc._tile_sem_poison_stack.pop()
        assert popped is self._sem_poison
        nc.clear_and_free_semaphores(list(self.sems.allocated().values()))
        nc.all_engine_barrier()

    TileContext._drain_and_barrier = _drain_and_barrier

    f32 = mybir.dt.float32
    f16 = mybir.dt.float16
    i32 = mybir.dt.int32
    AF = mybir.ActivationFunctionType
    OP = mybir.AluOpType
    AX = mybir.AxisListType

    nc = bass.Bass()
    X = nc.declare_dram_parameter("x", [TOK, D], f32, isOutput=False)
    KB = nc.declare_dram_parameter("kb16", [128, TOK], f16, isOutput=False)
    KV = nc.declare_dram_parameter("kvec", [TOK, 1], f32, isOutput=False)
    I2P = nc.declare_dram_parameter("inv2pi_e", [128, D * 4 * TT], f16, isOutput=False)
    PE_ = nc.declare_dram_parameter("P_e", [128, D * 4 * TT], f16, isOutput=False)
    ISM = nc.declare_dram_parameter("invsm", [128, SMALL], f32, isOutput=False)
    PSM = nc.declare_dram_parameter("Psm", [128, SMALL], f16, isOutput=False)
    MJO = nc.declare_dram_parameter("Mjo", [D, D], f32, isOutput=False)
    RJO = nc.declare_dram_parameter("Rjo", [D, D], f32, isOutput=False)
    BM = nc.declare_dram_parameter("biasM", [D, 1], f32, isOutput=False)
    BMB = nc.declare_dram_parameter("biasMb", [TOK, D], f32, isOutput=False)
    BRB = nc.declare_dram_parameter("biasRb", [TOK, D], f32, isOutput=False)
    REP = nc.declare_dram_parameter("rep", [D, 128], f32, isOutput=False)
    IDM = nc.declare_dram_parameter("idm", [128, 128], f32, isOutput=False)
    Y = nc.declare_dram_parameter("y", [TOK, D], f32, isOutput=True)

    with TileContext(nc) as tc:
        with (
            tc.tile_pool(name="const", bufs=1) as cp,
            tc.tile_pool(name="ang", bufs=3) as angp,
            tc.tile_pool(name="sino", bufs=2) as sinp,
            tc.tile_pool(name="prod", bufs=2) as prodp,
            tc.tile_pool(name="prow", bufs=4, space="PSUM") as prowp,
            tc.tile_pool(name="pprep", bufs=2, space="PSUM") as pprep,
            tc.tile_pool(name="dram", bufs=1, space="DRAM") as dramp,
        ):
            scratch = dramp.tile([16 * 4096], f32, tag="scr")
            # ---------------- load constants ----------------
            xs = cp.tile([TOK, D], f32, tag="xs")
            nc.sync.dma_start(out=xs[:], in_=X[:])
            kb = cp.tile([128, TOK], f16, tag="kb")
            nc.sync.dma_start(out=kb[:], in_=KB[:])
            kvec = cp.tile([TOK, 1], f32, tag="kvec")
            nc.sync.dma_start(out=kvec[:], in_=KV[:])
            i2p = cp.tile([128, D * 4 * TT], f16, tag="i2p")
            nc.sync.dma_start(out=i2p[:], in_=I2P[:])
            pe = cp.tile([128, D * 4 * TT], f16, tag="pe")
            nc.sync.dma_start(out=pe[:], in_=PE_[:])
            ism = cp.tile([128, SMALL], f32, tag="ism")
            nc.sync.dma_start(out=ism[:], in_=ISM[:])
            psm = cp.tile([128, SMALL], f16, tag="psm")
            nc.sync.dma_start(out=psm[:], in_=PSM[:])
            mjo = cp.tile([D, D], f32, tag="mjo")
            nc.sync.dma_start(out=mjo[:], in_=MJO[:])
            rjo = cp.tile([D, D], f32, tag="rjo")
            nc.sync.dma_start(out=rjo[:], in_=RJO[:])
            bm = cp.tile([D, 1], f32, tag="bm")
            nc.sync.dma_start(out=bm[:], in_=BM[:])
            bmb = cp.tile([TOK, D], f32, tag="bmb")
            nc.sync.dma_start(out=bmb[:], in_=BMB[:])
            brb = cp.tile([TOK, D], f32, tag="brb")
            nc.sync.dma_start(out=brb[:], in_=BRB[:])
            repm = cp.tile([D, 128], f32, tag="repm")
            nc.sync.dma_start(out=repm[:], in_=REP[:])
            idm = cp.tile([128, 128], f32, tag="idm")
            nc.sync.dma_start(out=idm[:], in_=IDM[:])

            bias_hp = cp.tile([128, 1], f32, tag="bias_hp")
            nc.vector.memset(bias_hp[:], math.pi / 2.0)
            bias_z = cp.tile([128, 1], f32, tag="bias_z")
            nc.vector.memset(bias_z[:], 0.0)

            # ---------------- LayerNorm (token-major, gamma/beta folded) ---
            rsum = cp.tile([TOK, 1], f32, tag="rsum")
            nc.vector.tensor_reduce(rsum[:], xs[:], AX.X, OP.add)
            mu = cp.tile([TOK, 1], f32, tag="mu")
            nc.vector.tensor_scalar(mu[:], rsum[:], 1.0 / D, None, OP.mult)
            cen = cp.tile([TOK, D], f32, tag="cen")
            nc.vector.tensor_scalar(cen[:], xs[:], mu[:], None, OP.subtract)
            sq = cp.tile([TOK, D], f32, tag="sq")
            nc.vector.tensor_tensor(sq[:], cen[:], cen[:], OP.mult)
            ssq = cp.tile([TOK, 1], f32, tag="ssq")
            nc.vector.tensor_reduce(ssq[:], sq[:], AX.X, OP.add)
            veps = cp.tile([TOK, 1], f32, tag="veps")
            nc.vector.tensor_scalar(veps[:], ssq[:], 1.0 / D, LN_EPS, OP.mult, OP.add)

            # rstd = 1/sqrt(veps): bit-hack seed + 3 Newton steps
            ti = cp.tile([TOK, 1], i32, tag="ti")
            nc.vector.tensor_scalar(
                ti[:], veps[:].bitcast(i32), 1, -1, OP.arith_shift_right,
                OP.bitwise_xor,
            )
            yr = cp.tile([TOK, 1], f32, tag="yr")
            nc.vector.tensor_scalar(
                yr[:].bitcast(i32), ti[:], 0x5F3759DF + 1, None, OP.add
            )
            hh = cp.tile([TOK, 1], f32, tag="hh")
            nc.vector.tensor_scalar(hh[:], veps[:], 0.5, None, OP.mult)
            for it in range(3):
                t1 = cp.tile([TOK, 1], f32, tag=f"nt1_{it}")
                nc.vector.tensor_tensor(t1[:], yr[:], yr[:], OP.mult)
                t2 = cp.tile([TOK, 1], f32, tag=f"nt2_{it}")
                nc.vector.tensor_tensor(t2[:], t1[:], hh[:], OP.mult)
                t3 = cp.tile([TOK, 1], f32, tag=f"nt3_{it}")
                nc.vector.tensor_scalar(t3[:], t2[:], 1.5, -1.0, OP.subtract, OP.mult)
                yn = cp.tile([TOK, 1], f32, tag=f"nt4_{it}")
                nc.vector.tensor_tensor(yn[:], yr[:], t3[:], OP.mult)
                yr = yn

            lnf = cp.tile([TOK, D], f32, tag="lnf")
            nc.vector.tensor_scalar(lnf[:], cen[:], yr[:], None, OP.mult)

            # ---------------- projections ----------------
            lnT_ps = pprep.tile([D, TOK], f32, tag="pp")
            nc.tensor.transpose(lnT_ps[:], lnf[:], idm[:])
            lnT = cp.tile([D, TOK], f32, tag="lnT")
            nc.vector.tensor_copy(lnT[:], lnT_ps[:])

            # xpT[o, t] = sum_j Mjo[j,o] * lnT[j,t] + biasM[o]
            xpT_ps = pprep.tile([D, TOK], f32, tag="pp")
            nc.tensor.matmul(xpT_ps[:], mjo[:], lnT[:], start=True, stop=True)
            xpT = cp.tile([D, TOK], f32, tag="xpT")
            nc.vector.tensor_scalar(xpT[:], xpT_ps[:], bm[:], None, OP.add)

            # xprep16[p, t] = xpT[p>>1, t] in fp16
            xpr_ps = pprep.tile([128, TOK], f32, tag="pp")
            nc.tensor.matmul(xpr_ps[:], repm[:], xpT[:], start=True, stop=True)
            xprep = cp.tile([128, TOK], f16, tag="xprep")
            nc.vector.tensor_copy(xprep[:], xpr_ps[:])

            # res_tm[t, o] = sum_j lnT[j,t] * Rjo[j,o]  (+ biasR)
            res_ps = pprep.tile([TOK, D], f32, tag="pp")
            nc.tensor.matmul(res_ps[:], lnT[:], rjo[:], start=True, stop=True)
            res_tm = cp.tile([TOK, D], f32, tag="res_tm")
            nc.vector.tensor_tensor(res_tm[:], res_ps[:], brb[:], OP.add)

            # xp_tm[t, o] (token-major xproj, for the small-p pass)
            xp_ps2 = pprep.tile([TOK, D], f32, tag="pp")
            nc.tensor.matmul(xp_ps2[:], lnT[:], mjo[:], start=True, stop=True)
            xp_tm = cp.tile([TOK, D], f32, tag="xp_tm")
            nc.vector.tensor_tensor(xp_tm[:], xp_ps2[:], bmb[:], OP.add)
            # xp4[t, (i4, j64)] = xp_tm[t, j] tiled 4x, fp16
            xp4 = cp.tile([TOK, 4 * D], f16, tag="xp4")
            nc.vector.tensor_copy(
                xp4[:].rearrange("p (a b) -> p a b", a=4),
                xp_tm[:].unsqueeze(1).broadcast_to([TOK, 4, D]),
            )

            # ---------------- small-p pass (token-major, i<4) --------------
            # heavy elementwise on GpSimd; sin on ScalarE; rest on DVE
            uu = cp.tile([128, SMALL], f32, tag="uu")
            nc.gpsimd.tensor_scalar(uu[:], ism[:], kvec[:], 0.25, OP.mult, OP.add)
            rr = cp.tile([128, SMALL], f32, tag="rr")
            nc.gpsimd.tensor_scalar(rr[:], uu[:], RND_C, RND_C, OP.add, OP.subtract)
            ff = cp.tile([128, SMALL], f32, tag="ff")
            nc.gpsimd.tensor_tensor(ff[:], uu[:], rr[:], OP.subtract)

            # ---------------- main loop: 8 tiles x 16 tokens ---------------
            out_sb = cp.tile([TOK, D], f32, tag="out_sb")
            s0 = cp.tile([128, SMALL], f16, tag="s0")
            for tau in range(NTILE):
                t0 = tau * TT
                ang = angp.tile([128, D * 4 * TT], f16, tag="ang")
                kb_b = (
                    kb[:, t0 : t0 + TT]
                    .unsqueeze(1)
                    .unsqueeze(1)
                    .broadcast_to([128, D, 4, TT])
                )
                i2p_v = i2p[:].rearrange("p (i glo t) -> p i glo t", glo=4, t=TT)
                ang_v = ang[:].rearrange("p (i glo t) -> p i glo t", glo=4, t=TT)
                eng = nc.gpsimd if tau in POOL_ANG else nc.vector
                eng.tensor_tensor(ang_v, kb_b, i2p_v, OP.mult)

                # sin in natural (i,glo,t) layout
                sino = sinp.tile([128, D * 4 * TT], f16, tag="sino")
                nc.scalar.activation(
                    sino[:], ang[:], AF.Sin, bias=bias_hp[:], scale=1.0
                )
                # small-p sin placed mid-stream on the Scalar queue (its ff
                # input lands at ~12us; tiles 0..3 keep ScalarE busy till then)
                if tau == 4:
                    nc.scalar.activation(
                        s0[:], ff[:], AF.Sin, bias=bias_z[:], scale=TWO_PI
                    )

                prodx = prodp.tile([128, D * 4 * TT], f16, tag="prodx")
                nc.vector.tensor_tensor(prodx[:], sino[:], pe[:], OP.mult)

                # moving view: per glo, columns ordered (t outer, i inner)
                pv = prodx[:].rearrange("p (i glo t) -> p glo t i", glo=4, t=TT)
                for h in range(2):
                    g = 2 * tau + h
                    rows = prowp.tile([8, 8 * D], f32, tag="rows")
                    for glo in range(4):
                        nc.tensor.matmul(
                            rows[:],
                            xprep[:, t0 + h * 8 : t0 + h * 8 + 8],
                            pv[:, glo, h * 8 : (h + 1) * 8, :],
                            start=(glo == 0),
                            stop=(glo == 3),
                        )
                    # PSUM -> SBUF copy (engine round-robined), then dump to
                    # HBM and gather the diagonal blocks back: flat offset
                    # t*576 + i walks out[t, t*64+i]
                    rsb = cp.tile([8, 8 * D], f32, tag=f"rsb{g}")
                    if g % 2 == 0:
                        nc.scalar.copy(rsb[:], rows[:])
                    else:
                        nc.vector.tensor_copy(rsb[:], rows[:])
                    nc.sync.dma_start(
                        out=scratch[g * 4096 : (g + 1) * 4096].rearrange(
                            "(t n) -> t n", n=8 * D
                        ),
                        in_=rsb[:],
                    )
                    diag = AP(scratch[:].tensor, g * 4096, [[576, 8], [1, D]])
                    nc.sync.dma_start(
                        out=out_sb[g * 8 : (g + 1) * 8, :], in_=diag
                    )

            # ---------------- small-p tail ----------------
            prod0 = cp.tile([128, SMALL], f16, tag="prod0")
            nc.gpsimd.tensor_tensor(prod0[:], s0[:], psm[:], OP.mult)
            rg = cp.tile([128, 256], f16, tag="rg")
            with nc.allow_low_precision(reason="8-term fp16 partial sums"):
                nc.vector.tensor_reduce(
                    rg[:], prod0[:].rearrange("p (a b) -> p a b", b=NB), AX.X, OP.add
                )
            rgx = cp.tile([128, 256], f16, tag="rgx")
            nc.vector.tensor_tensor(rgx[:], rg[:], xp4[:], OP.mult)
            nksm = cp.tile([128, 4], f32, tag="nksm")
            nc.vector.tensor_reduce(
                nksm[:], rgx[:].rearrange("p (a b) -> p a b", b=D), AX.X, OP.add
            )

            # ---------------- combine + output ----------------
            nc.vector.tensor_tensor(out_sb[:], out_sb[:], res_tm[:], OP.add)
            nc.vector.tensor_tensor(
                out_sb[:, 0:4], out_sb[:, 0:4], nksm[:], OP.add
            )
            nc.sync.dma_start(out=Y[:], in_=out_sb[:])

    if split:
        _split_waits(nc)
    return nc


def _host_prep(x, k, M, R, P, gamma, beta, periods):
    f16 = np.float16
    xf = np.ascontiguousarray(x, dtype=np.float32).reshape(B * S, D)
    kf = np.ascontiguousarray(k, dtype=np.float32).reshape(B * S)
    Mf = np.asarray(M, dtype=np.float32)
    Rf = np.asarray(R, dtype=np.float32)
    Pf = np.asarray(P, dtype=np.float32).reshape(-1)
    gf = np.asarray(gamma, dtype=np.float32).reshape(D)
    bf = np.asarray(beta, dtype=np.float32).reshape(D)
    pf = np.asarray(periods, dtype=np.float32).reshape(-1)

    # parameter preprocessing (k/x independent)
    per_pig = pf.reshape(D, 128, 4).transpose(1, 0, 2)  # [p, i, glo]
    inv2pi = (TWO_PI / per_pig).astype(np.float32)
    inv2pi[:, :4, :] = 0.0
    inv2pi_e = np.ascontiguousarray(
        np.broadcast_to(inv2pi[:, :, :, None], (128, D, 4, TT)).astype(f16)
    ).reshape(128, D * 4 * TT)

    P_pig = Pf.reshape(D, 128, 4).transpose(1, 0, 2).copy()  # [p, i, glo]
    P_pig[:, :4, :] = 0.0
    P_e = np.ascontiguousarray(
        np.broadcast_to(P_pig[:, :, :, None], (128, D, 4, TT)).astype(f16)
    ).reshape(128, D * 4 * TT)

    invsm = np.ascontiguousarray(
        np.broadcast_to((1.0 / pf[:SMALL])[None, :], (128, SMALL))
    ).astype(np.float32)
    Psm = np.ascontiguousarray(
        np.broadcast_to(Pf[:SMALL][None, :], (128, SMALL))
    ).astype(f16)

    Mjo = np.ascontiguousarray(gf[:, None] * Mf.T).astype(np.float32)
    Rjo = np.ascontiguousarray(gf[:, None] * Rf.T).astype(np.float32)
    biasM = (bf @ Mf.T).astype(np.float32)
    biasR = (bf @ Rf.T).astype(np.float32)
    biasMb = np.ascontiguousarray(
        np.broadcast_to(biasM[None, :], (TOK, D))
    ).astype(np.float32)
    biasRb = np.ascontiguousarray(
        np.broadcast_to(biasR[None, :], (TOK, D))
    ).astype(np.float32)

    rep = np.zeros((D, 128), np.float32)
    rep[np.arange(128) // 2, np.arange(128)] = 1.0
    idm = np.eye(128, dtype=np.float32)

    in_maps = []
    for core in range(N_CORES):
        sl = slice(core * TOK, (core + 1) * TOK)
        kc = kf[sl]
        in_maps.append(
            {
                "x": xf[sl],
                "kb16": np.ascontiguousarray(
                    np.broadcast_to(kc[None, :], (128, TOK))
                ).astype(f16),
                "kvec": kc[:, None].astype(np.float32).copy(),
                "inv2pi_e": inv2pi_e,
                "P_e": P_e,
                "invsm": invsm,
                "Psm": Psm,
                "Mjo": Mjo,
                "Rjo": Rjo,
                "biasM": biasM[:, None].copy(),
                "biasMb": biasMb,
                "biasRb": biasRb,
                "rep": rep,
                "idm": idm,
            }
        )
    return in_maps


def kernel(x, k, M, R, P, gamma, beta, periods):
    from concourse.bass_utils import run_bass_kernel_spmd

    if "nc" not in _CACHE:
        _CACHE["nc"] = _build_program()
    nc = _CACHE["nc"]

    in_maps = _host_prep(x, k, M, R, P, gamma, beta, periods)
    _CACHE["in_maps"] = in_maps
    res = run_bass_kernel_spmd(nc, in_maps, core_ids=list(range(N_CORES)))
    out = np.concatenate([res.results[c]["y"] for c in range(N_CORES)], axis=0)
    return out.reshape(B, S, D)
